# revision 27
# baseline (speedup 1.0000x reference)
"""Trainium2 Bass kernel for nn_CaptionEmbedding (ragged double-GRU with
attention gating).

Strategy: data-parallel over batch across 8 cores (strided over the
length-sorted order so every core gets a balanced length mix). Per core a
fully-unrolled 20-step recurrence.

Precision plan (validated vs fp32 reference in numpy, rel ~7e-3):
  - fp8(e4m3) + MatmulPerfMode.DoubleRow (2 k-tiles/instr, 0.5 cyc/row)
    for the error-tolerant matmuls: Whh, Wih, cWhh, Wh. Operands carry
    power-of-2 scales; descales fold into activation `scale` params.
  - fp16 for the output-sensitive matmuls: cWih, Wl, Wf (their operands
    feed outs/alphas nearly unfiltered).
  - ALL weights resident in SBUF (~20MB): no per-step weight streaming.

Scales: s_x=16 (x fp8), s_h=64 (h1/h2 fp8), weights: Wih x256, Whh x64,
Wh x512, cWhh x64, cWih(fp16) x256. PSUM raw scales: w-GRU gates 4096,
attention Bp 32768, c-GRU gates 4096 (cWih-fp16 and cWhh-fp8 products
accumulate in the same PSUM at matching scale). Wl/Wf/joint/grc/att true.
"""
import numpy as np
import ml_dtypes

import concourse.bass as bass
import concourse.mybir as mybir
import concourse.tile as tile
from concourse.bass_utils import run_bass_kernel_spmd
import concourse.mybir as _mybir

B, VD, QD, HD, L = 512, 2048, 1024, 1024, 20
NCORES, S = 8, 64
F32, F16 = mybir.dt.float32, mybir.dt.float16
F8 = mybir.dt.float8e4
E4NP = ml_dtypes.float8_e4m3
DR = mybir.MatmulPerfMode.DoubleRow
Sig = mybir.ActivationFunctionType.Sigmoid
Tanh = mybir.ActivationFunctionType.Tanh
Relu = mybir.ActivationFunctionType.Relu
Copy = mybir.ActivationFunctionType.Copy
Mult = mybir.AluOpType.mult
Add = mybir.AluOpType.add

S_X = 16.0      # caption fp8 scale
S_H = 64.0      # h1/h2 fp8 scale
S_WIH = 256.0   # -> gi psum raw 4096
S_WHH = 64.0    # -> gh psum raw 4096
S_WH = 512.0    # -> Bp psum raw 32768
S_CWHH = 64.0   # -> ghc psum raw 4096
S_CWIH = 256.0  # fp16 weights prescaled; (16*cin)*(256*W) -> raw 4096
S1 = 4096.0     # w-GRU / c-GRU gate psum descale
S3 = 32768.0    # attention Bp descale

_MAX_WAITS = 1
_wait_ctr = [0]


def _dedupe_ldw(nc):
    """Tile legalization emits one InstLdweights per matmul; consecutive
    matmuls over the same stationary reload identical weights. Drop exact
    duplicates (no sync side effects) so the PE streams back-to-back."""
    import concourse.mybir as mb

    dropped = 0
    for fn in nc.m.functions:
        for bb in fn.blocks:
            out = []
            last = {}
            for inst in bb.instructions:
                nm = type(inst).__name__
                if nm == "InstLdweights":
                    si = inst.sync_info
                    pos = (
                        tuple(getattr(inst, "tile_position", None) or (-1,)),
                        tuple(getattr(inst, "tile_size", None) or (-1,)),
                    )
                    key = (
                        str(inst.ins[0]),
                        bool(getattr(inst, "is_transpose", False)),
                        str(getattr(inst, "perf_mode", None)),
                    )
                    clean = not (si and (si.on_wait or si.on_update))
                    if clean and last.get(pos) == key:
                        dropped += 1
                        continue
                    last[pos] = key
                elif nm == "InstMatmult":
                    pass
                elif inst.engine == mb.EngineType.PE:
                    last.clear()
                out.append(inst)
            if len(out) != len(bb.instructions):
                bb.instructions[:] = out
    return dropped


def _split_waits(nc):
    # container neuronxcc rejects >= 2 sync waits on one instruction; move
    # extras onto same-engine nops spliced just before it
    for fn in nc.m.functions:
        for bb in fn.blocks:
            out = []
            for inst in bb.instructions:
                si = inst.sync_info
                waits = list(si.on_wait) if si and si.on_wait else []
                if len(waits) > _MAX_WAITS:
                    extra, keep = waits[:-_MAX_WAITS], waits[-_MAX_WAITS:]
                    for i in range(0, len(extra), _MAX_WAITS):
                        _wait_ctr[0] += 1
                        nop = _mybir.InstNoOp(
                            name=f"waitsplit_nop_{_wait_ctr[0]}", ins=[], outs=[]
                        )
                        nop.engine = inst.engine
                        nop.sync_info = _mybir.SyncInfo(
                            on_wait=extra[i : i + _MAX_WAITS], on_update=[]
                        )
                        nc.register_instruction(nop)
                        out.append(nop)
                    si.on_wait = keep
                out.append(inst)
            if len(out) != len(bb.instructions):
                bb.instructions[:] = out


def _kt_slice(tT, kt):
    # stationary [128, 64] for feature ktile kt from a transposed
    # [128, 4, 128] tile: tT[p, j, q] = stacked[q, j*128 + p]
    hi, j = kt // 4, kt % 4
    return tT[:, j, 64 * hi : 64 * hi + 64]


def _kp_slice(tT, kp):
    # DoubleRow stationary [128, 2, 64] for ktile-pair kp from a
    # transposed state tile; feature = 512*(kp//2) + 256*(kp%2) + 128*i + p
    j0 = 2 * (kp % 2)
    q0 = 64 * (kp // 2)
    return tT[:, j0 : j0 + 2, q0 : q0 + 64]


def _build():
    """Trace the per-core program (identical for all cores; SPMD)."""
    nc = bass.Bass("TRN2", dynamic_dma_scratch_size=64)
    di = {}
    inputs = [
        ("vT", [128, 16, S], F16),
        ("qT", [128, 8, S], F16),
        ("xT", [L, 128, 8, S], F8),
        ("wvT", [16, 128, HD], F16),
        ("wqT", [8, 128, HD], F16),
        ("wih8", [4, 128, 2, 3 * HD], F8),
        ("whh8", [4, 128, 2, 3 * HD], F8),
        ("wh8", [4, 128, 2, HD], F8),
        ("wcwh8", [4, 128, 2, 3 * HD], F8),
        ("wcih16", [8, 128, 3 * HD], F16),
        ("wl16", [8, 128, HD], F16),
        ("wf16", [8, 128, HD], F16),
    ]
    for name, shape, dt in inputs:
        di[name] = nc.dram_tensor(name, shape, dt, kind="ExternalInput")
    outs_d = nc.dram_tensor("outs", [L, 128, 512], F32, kind="ExternalOutput")
    alph_d = nc.dram_tensor("alph", [L, 128, 512], F16, kind="ExternalOutput")

    with tile.TileContext(nc) as tc:
        _trace(nc, tc, di, outs_d, alph_d)
    _dedupe_ldw(nc)
    _split_waits(nc)
    return nc


def _mm_gate16(nc, psum, lhsT, w_ap, c0, start, stop):
    """fp16: one ktile's pair of matmuls for a 1024-wide gate at weight
    cols [c0, c0+1024): the two halves use the PE's two column groups
    concurrently (stacked [128,512] psum)."""
    nc.tensor.matmul(psum[0:64, :], lhsT, w_ap[:, c0 : c0 + 512],
                     start=start, stop=stop)
    nc.tensor.matmul(psum[64:128, :], lhsT, w_ap[:, c0 + 512 : c0 + 1024],
                     start=start, stop=stop)


def _mm_gate8(nc, ab, lhsT, w_t, kp, c0, start, stop):
    """fp8 DoubleRow: one ktile-PAIR's matmuls for a 1024-wide gate.
    DR outputs must sit at PSUM base partition 0, hence [64,512] pairs.
    Weights carry the k-pair dim INNERMOST (2 fp8 adjacent per column)."""
    nc.tensor.matmul(ab[0], lhsT, w_t[:, kp, :, c0 : c0 + 512],
                     start=start, stop=stop, perf_mode=DR)
    nc.tensor.matmul(ab[1], lhsT, w_t[:, kp, :, c0 + 512 : c0 + 1024],
                     start=start, stop=stop, perf_mode=DR)


def _trace(nc, tc, di, outs_d, alph_d):
    import contextlib

    ctx = contextlib.ExitStack()
    with ctx:
        work = ctx.enter_context(tc.tile_pool(name="work", bufs=1))
        res1 = ctx.enter_context(tc.tile_pool(name="res1", bufs=1))

        # ---- resident weights (fp8 DR layouts + fp16) ----
        wih_t = res1.tile([128, 4, 2, 3 * HD], F8, tag="wih8")
        wh_t = res1.tile([128, 4, 2, HD], F8, tag="wh8")
        wl_t = res1.tile([128, 8, HD], F16, tag="wl16")
        wcih_t = res1.tile([128, 8, 3 * HD], F16, tag="wcih16")
        wf_t = res1.tile([128, 8, HD], F16, tag="wf16")
        whh_t = res1.tile([128, 4, 2, 3 * HD], F8, tag="whh8")
        wcwh_t = res1.tile([128, 4, 2, 3 * HD], F8, tag="wcwh8")
        # DMA order = first-use order
        for kp in range(4):
            nc.sync.dma_start(out=wih_t[:, kp], in_=di["wih8"][kp])
        for kp in range(4):
            nc.sync.dma_start(out=wh_t[:, kp], in_=di["wh8"][kp])
        for kt in range(8):
            nc.sync.dma_start(out=wl_t[:, kt], in_=di["wl16"][kt])
        for kt in range(8):
            nc.sync.dma_start(out=wcih_t[:, kt], in_=di["wcih16"][kt])
        for kt in range(8):
            nc.sync.dma_start(out=wf_t[:, kt], in_=di["wf16"][kt])
        for kp in range(4):
            nc.sync.dma_start(out=whh_t[:, kp], in_=di["whh8"][kp])
        for kp in range(4):
            nc.sync.dma_start(out=wcwh_t[:, kp], in_=di["wcwh8"][kp])

        # ---- small persistent tiles ----
        pvq_t = work.tile([128, 512], F32, tag="pvq")  # true scale

        ctr = [0]

        def wtile(shape, dt, tag, bufs):
            def mk():
                ctr[0] += 1
                return work.tile(shape, dt, tag=tag, bufs=bufs,
                                 name=f"{tag}_{ctr[0]}")
            return mk

        t_xt = wtile([128, 8, S], F8, "xt", 3)
        t_gi = wtile([128, 512], F16, "gi", 6)     # raw S1
        t_ec = wtile([128, 512], F16, "ec", 3)     # cWhh products, raw S1
        t_ag = wtile([128, 512], F16, "ag", 3)     # Whh products, raw S1
        t_g = wtile([128, 512], F16, "g", 5)
        t_h1 = wtile([128, 512], F16, "h1", 2)     # true
        t_h1T = wtile([128, 4, 128], F8, "h1T", 2)
        t_h2 = wtile([128, 512], F16, "h2", 2)
        t_h2T = wtile([128, 4, 128], F8, "h2T", 2)
        t_h2n = wtile([128, 512], F32, "h2n", 1)
        t_ja = wtile([128, 512], F16, "ja", 1)     # true (STT descaled)
        t_att = wtile([128, 512], F16, "att", 1)
        t_attT = wtile([128, 4, 128], F16, "attT", 2)
        t_jrl = wtile([128, 512], F16, "jrl", 1)
        t_jT = wtile([128, 4, 128], F16, "jT", 2)
        t_grc = wtile([128, 512], F16, "grc", 1)
        t_grcT = wtile([128, 4, 128], F16, "grcT", 2)
        t_cinT = wtile([128, 4, 128], F16, "cinT", 1)

        psum = ctx.enter_context(tc.tile_pool(name="psum", bufs=1, space="PSUM"))

        def ptile():
            # one [64,512] fp32 bank-tile (DR-legal: base partition 0)
            ctr[0] += 1
            return psum.tile([64, 512], F32, tag="ps", name=f"ps_{ctr[0]}",
                             bufs=4)

        def pgate():
            return (ptile(), ptile())

        def ptile128():
            # [128,512] stacked psum for fp16 two-column-group matmuls
            ctr[0] += 1
            return psum.tile([128, 512], F32, tag="psf",
                             name=f"psf_{ctr[0]}", bufs=3)

        def pttile16():
            ctr[0] += 1
            return psum.tile([128, 512], F16, tag="psT", name=f"psT_{ctr[0]}",
                             bufs=1)

        def halves(x):
            # (lo, hi) partition halves of a stacked [128, 512] AP
            return x[0:64, :], x[64:128, :]

        ident16 = work.tile([128, 128], F16, tag="ident16")
        from concourse.masks import make_identity
        make_identity(nc, ident16)

        def pe_transpose16(dstT, src):
            pt = pttile16()
            for j in range(4):
                nc.tensor.transpose(
                    pt[:, 128 * j : 128 * (j + 1)],
                    src[:, 128 * j : 128 * (j + 1)],
                    ident16,
                )
            nc.vector.tensor_copy(
                out=dstT.rearrange("p j q -> p (j q)"), in_=pt
            )

        def pe_transpose8(dstT, src16, scale):
            # fp16 transpose, then quantize to fp8 (x scale) in the
            # PSUM->SBUF copy (fp8 PE-transpose needs stride-2 out APs)
            pt = pttile16()
            for j in range(4):
                nc.tensor.transpose(
                    pt[:, 128 * j : 128 * (j + 1)],
                    src16[:, 128 * j : 128 * (j + 1)],
                    ident16,
                )
            nc.vector.tensor_scalar(
                dstT.rearrange("p j q -> p (j q)"), pt, scale, None, Mult
            )

        # ---- gi (w-GRU input projections), fp8 DR, computed ahead ----
        gi_tiles = {}
        xt_tiles = {}

        def load_xt(t):
            xt = t_xt()
            nc.scalar.dma_start(out=xt, in_=di["xT"][t])
            xt_tiles[t] = xt

        def gi_gate(u, g):
            # one gate (4 kp DR accum) -> stacked SBUF raw-S1 fp16 copy
            if u >= L:
                return
            xt = xt_tiles[u]
            GP = pgate()
            for kp in range(4):
                lhsT = xt[:, 2 * kp : 2 * kp + 2, :]
                _mm_gate8(nc, GP, lhsT, wih_t, kp, g * HD,
                          kp == 0, kp == 3)
            gt = t_gi()
            glo, ghi = halves(gt)
            nc.vector.tensor_copy(out=glo, in_=GP[0])
            nc.vector.tensor_copy(out=ghi, in_=GP[1])
            gi_tiles.setdefault(u, []).append(gt)

        # ---- prologue: pvq = v @ Wv.T + q @ Wq.T (fp16, true scale) ----
        with tc.tile_pool(name="pre", bufs=1) as pre:
            v_t = pre.tile([128, 16, S], F16, tag="v")
            q_t = pre.tile([128, 8, S], F16, tag="q")
            nc.scalar.dma_start(out=v_t, in_=di["vT"][:])
            nc.scalar.dma_start(out=q_t, in_=di["qT"][:])
            pv = ptile128()
            for kt in range(16):
                wc = pre.tile([128, HD], F16, tag="wvq", bufs=2)
                nc.scalar.dma_start(out=wc, in_=di["wvT"][kt])
                nc.tensor.matmul(pv[0:64], v_t[:, kt, :], wc[:, 0:512],
                                 start=(kt == 0), stop=False)
                nc.tensor.matmul(pv[64:128], v_t[:, kt, :], wc[:, 512:1024],
                                 start=(kt == 0), stop=False)
            for kt in range(8):
                wc = pre.tile([128, HD], F16, tag="wvq", bufs=2)
                nc.scalar.dma_start(out=wc, in_=di["wqT"][kt])
                nc.tensor.matmul(pv[0:64], q_t[:, kt, :], wc[:, 0:512],
                                 start=False, stop=(kt == 7))
                nc.tensor.matmul(pv[64:128], q_t[:, kt, :], wc[:, 512:1024],
                                 start=False, stop=(kt == 7))
            nc.vector.tensor_copy(out=pvq_t, in_=pv)

            load_xt(0)
            load_xt(1)
            for g in range(3):
                gi_gate(0, g)
            for g in range(3):
                gi_gate(1, g)

        h1_prev = None
        h1T_prev = None
        h2_prev = None
        h2T_prev = None

        # ---- main loop ----
        for t in range(L):
            if t + 2 < L:
                load_xt(t + 2)

            giR, giZ, giIN = gi_tiles.pop(t)
            xt = xt_tiles.pop(t)

            if t > 0:
                # --- interleaved c-GRU / w-GRU hidden projections (fp8 DR);
                # every product pair is copied to stacked SBUF so PSUM
                # pairs recycle fast ---
                ec_sb = []
                ag_sb = []
                for g in range(3):
                    EC = pgate()
                    for kp in range(4):
                        _mm_gate8(nc, EC, _kp_slice(h2T_prev, kp), wcwh_t,
                                  kp, g * HD, kp == 0, kp == 3)
                    ecs = t_ec()
                    elo, ehi = halves(ecs)
                    nc.vector.tensor_copy(out=elo, in_=EC[0])
                    nc.vector.tensor_copy(out=ehi, in_=EC[1])
                    ec_sb.append(ecs)
                    WG = pgate()
                    for kp in range(4):
                        _mm_gate8(nc, WG, _kp_slice(h1T_prev, kp), whh_t,
                                  kp, g * HD, kp == 0, kp == 3)
                    ags = t_ag()
                    alo, ahi = halves(ags)
                    nc.vector.tensor_copy(out=alo, in_=WG[0])
                    nc.vector.tensor_copy(out=ahi, in_=WG[1])
                    ag_sb.append(ags)
                Rw, Zw, HNw = ag_sb
                Rcs, Zcs, HNcs = ec_sb

                # --- w-GRU combine (raw S1 -> true); z-branch on gpsimd ---
                rs = t_g()
                nc.vector.tensor_add(out=rs, in0=Rw, in1=giR)
                rw = t_g()
                nc.scalar.activation(out=rw, in_=rs, func=Sig, scale=1.0 / S1)
                t1 = t_g()
                nc.vector.tensor_mul(out=t1, in0=rw, in1=HNw)
                t2 = t_g()
                nc.vector.tensor_add(out=t2, in0=t1, in1=giIN)
                nw = t_g()
                nc.scalar.activation(out=nw, in_=t2, func=Tanh, scale=1.0 / S1)
                zs = t_g()
                nc.gpsimd.tensor_add(out=zs, in0=Zw, in1=giZ)
                zw = t_g()
                nc.scalar.activation(out=zw, in_=zs, func=Sig, scale=1.0 / S1)
                f1 = t_g()
                nc.gpsimd.tensor_scalar(f1, zw, -1.0, 1.0, Mult, Add)
                f2 = t_g()
                nc.vector.tensor_mul(out=f2, in0=f1, in1=nw)
                f3 = t_g()
                nc.gpsimd.tensor_mul(out=f3, in0=zw, in1=h1_prev)
                h1_new = t_h1()
                nc.vector.tensor_add(out=h1_new, in0=f2, in1=f3)
            else:
                Rcs = Zcs = HNcs = None
                rw = t_g()
                nc.scalar.activation(out=rw, in_=giR, func=Sig, scale=1.0 / S1)
                nw = t_g()
                nc.scalar.activation(out=nw, in_=giIN, func=Tanh,
                                     scale=1.0 / S1)
                zw = t_g()
                nc.scalar.activation(out=zw, in_=giZ, func=Sig, scale=1.0 / S1)
                f1 = t_g()
                nc.vector.tensor_scalar(f1, zw, -1.0, 1.0, Mult, Add)
                h1_new = t_h1()
                nc.vector.tensor_mul(out=h1_new, in0=f1, in1=nw)

            h1T_new = t_h1T()
            pe_transpose8(h1T_new, h1_new, S_H)

            # --- attention: joint = relu(pvq + h1 @ Wh.T) ---
            Bp = pgate()
            for kp in range(4):
                _mm_gate8(nc, Bp, _kp_slice(h1T_new, kp), wh_t, kp, 0,
                          kp == 0, kp == 3)
            ja = t_ja()
            jalo, jahi = halves(ja)
            pqlo, pqhi = halves(pvq_t)
            nc.vector.scalar_tensor_tensor(
                out=jalo, in0=Bp[0], scalar=1.0 / S3, in1=pqlo,
                op0=Mult, op1=Add)
            nc.vector.scalar_tensor_tensor(
                out=jahi, in0=Bp[1], scalar=1.0 / S3, in1=pqhi,
                op0=Mult, op1=Add)
            jrl = t_jrl()
            nc.scalar.activation(out=jrl, in_=ja, func=Relu)
            gi_gate(t + 2, 0)
            jT = t_jT()
            pe_transpose16(jT, jrl)

            # --- att = sigmoid(joint @ Wl.T)  (fp16, true) ---
            Cp = ptile128()
            for kt in range(8):
                _mm_gate16(nc, Cp, _kt_slice(jT, kt), wl_t[:, kt, :], 0,
                           kt == 0, kt == 7)
            att = t_att()
            nc.scalar.activation(out=att, in_=Cp, func=Sig)
            nc.sync.dma_start(out=alph_d[t], in_=att)
            attT = t_attT()
            pe_transpose16(attT, att)

            gi_gate(t + 2, 1)

            # --- cin = att * x (feature-major, raw S_X) ---
            cinT = t_cinT()
            xt_r = xt.rearrange("p (hi j) s -> p j hi s", hi=2, j=4)
            nc.vector.tensor_mul(
                out=cinT.rearrange("p j (hi s) -> p j hi s", hi=2),
                in0=attT.rearrange("p j (hi s) -> p j hi s", hi=2),
                in1=xt_r,
            )

            # --- c-GRU input projections (fp16) ---
            RcI, ZcI, INc = ptile128(), ptile128(), ptile128()
            for kt in range(8):
                lhsT = _kt_slice(cinT, kt)
                st, sp = kt == 0, kt == 7
                _mm_gate16(nc, RcI, lhsT, wcih_t[:, kt, :], 0, st, sp)
                _mm_gate16(nc, ZcI, lhsT, wcih_t[:, kt, :], HD, st, sp)
                _mm_gate16(nc, INc, lhsT, wcih_t[:, kt, :], 2 * HD, st, sp)

            gi_gate(t + 2, 2)

            # --- c-GRU combine (raw S1 -> true); z-branch on gpsimd ---
            if t > 0:
                rsc = t_g()
                nc.vector.tensor_add(out=rsc, in0=RcI, in1=Rcs)
                rc = t_g()
                nc.scalar.activation(out=rc, in_=rsc, func=Sig, scale=1.0 / S1)
                t1c = t_g()
                nc.vector.tensor_mul(out=t1c, in0=rc, in1=HNcs)
                t2c = t_g()
                nc.vector.tensor_add(out=t2c, in0=t1c, in1=INc)
                ncg = t_g()
                nc.scalar.activation(out=ncg, in_=t2c, func=Tanh,
                                     scale=1.0 / S1)
                zsc = t_g()
                nc.vector.tensor_add(out=zsc, in0=ZcI, in1=Zcs)
                zc = t_g()
                nc.scalar.activation(out=zc, in_=zsc, func=Sig, scale=1.0 / S1)
                g1 = t_g()
                nc.gpsimd.tensor_scalar(g1, zc, -1.0, 1.0, Mult, Add)
                g2 = t_g()
                nc.vector.tensor_mul(out=g2, in0=g1, in1=ncg)
                g3 = t_g()
                nc.gpsimd.tensor_mul(out=g3, in0=zc, in1=h2_prev)
                grc = t_grc()
                nc.vector.tensor_add(out=grc, in0=g2, in1=g3)
            else:
                ncg = t_g()
                nc.scalar.activation(out=ncg, in_=INc, func=Tanh,
                                     scale=1.0 / S1)
                zc = t_g()
                nc.scalar.activation(out=zc, in_=ZcI, func=Sig, scale=1.0 / S1)
                g1 = t_g()
                nc.vector.tensor_scalar(g1, zc, -1.0, 1.0, Mult, Add)
                grc = t_grc()
                nc.vector.tensor_mul(out=grc, in0=g1, in1=ncg)
            grcT = t_grcT()
            pe_transpose16(grcT, grc)

            # --- h2n = gru_c @ Wf.T (fp16, true) ---
            Fp = ptile128()
            for kt in range(8):
                _mm_gate16(nc, Fp, _kt_slice(grcT, kt), wf_t[:, kt, :], 0,
                           kt == 0, kt == 7)
            h2n = t_h2n()
            nc.vector.tensor_copy(out=h2n, in_=Fp)
            nc.sync.dma_start(out=outs_d[t], in_=h2n)
            h2_new = t_h2()
            nc.scalar.activation(out=h2_new, in_=Fp, func=Copy)
            h2T_new = t_h2T()
            pe_transpose8(h2T_new, h2_new, S_H)

            h1_prev, h1T_prev = h1_new, h1T_new
            h2_prev, h2T_prev = h2_new, h2T_new


_CACHED = {}


def _get_nc():
    if "nc" not in _CACHED:
        _CACHED["nc"] = _build()
    return _CACHED["nc"]


def _wn(V, g):
    return V * (g / np.linalg.norm(V.astype(np.float64)).astype(np.float32))


def _plainT16(W, scale=1.0):
    # [out, in] -> [in//128, 128, out] fp16
    inf = W.shape[1]
    return np.ascontiguousarray(
        (W.T * scale).reshape(inf // 128, 128, W.shape[0])
    ).astype(np.float16)


def _pack8_state(W, scale):
    # [out, 1024] -> [4, 128, out, 2] e4m3 (k-pair dim innermost); feature
    # f at [kp, p, :, i] with f = 512*(kp//2) + 256*(kp%2) + 128*i + p
    # (matches _kp_slice on the transposed stacked state tiles)
    T = (W.T.astype(np.float32) * scale).reshape(2, 2, 2, 128, W.shape[0])
    T = np.ascontiguousarray(T.transpose(0, 1, 3, 2, 4).reshape(
        4, 128, 2, W.shape[0]))
    return T.astype(E4NP)


def _pack8_x(W, scale):
    # [out, 1024] -> [4, 128, out, 2] e4m3; f = 256*kp + 128*i + p
    # (matches xt[:, 2kp:2kp+2, :] stationaries)
    T = (W.T.astype(np.float32) * scale).reshape(4, 2, 128, W.shape[0])
    T = np.ascontiguousarray(T.transpose(0, 2, 1, 3))
    return T.astype(E4NP)


def _prep_in_maps(inp):
    cap_len = inp["cap_len"].astype(np.int32)
    order = np.argsort(-cap_len, kind="stable")

    for bname in ["av_b", "aq_b", "ah_b", "al_b", "fc_b",
                  "w_bih", "w_bhh", "c_bih", "c_bhh"]:
        assert not np.any(inp[bname]), f"nonzero bias {bname} unsupported"

    Wv = _wn(inp["av_V"], inp["av_g"])
    Wq = _wn(inp["aq_V"], inp["aq_g"])
    Wh = _wn(inp["ah_V"], inp["ah_g"])
    Wl = _wn(inp["al_V"], inp["al_g"])
    Wf = _wn(inp["fc_V"], inp["fc_g"])

    shared = dict(
        wvT=_plainT16(Wv), wqT=_plainT16(Wq),
        wih8=_pack8_x(inp["w_Wih"], S_WIH),
        whh8=_pack8_state(inp["w_Whh"], S_WHH),
        wh8=_pack8_state(Wh, S_WH),
        wcwh8=_pack8_state(inp["c_Whh"], S_CWHH),
        wcih16=_plainT16(inp["c_Wih"], S_CWIH),
        wl16=_plainT16(Wl), wf16=_plainT16(Wf),
    )

    v, q, caption = inp["v"], inp["q"], inp["caption"]
    in_maps = []
    for k in range(NCORES):
        pos = np.arange(S) * NCORES + k  # sorted positions of this core
        vk = v[pos].astype(np.float16)            # [S, VD]
        qk = q[pos].astype(np.float16)
        capk = (caption[order[pos]].astype(np.float32) * S_X)  # [S, L, QD]
        m = dict(shared)
        m["vT"] = np.ascontiguousarray(
            np.transpose(vk.T.reshape(16, 128, S), (1, 0, 2)))
        m["qT"] = np.ascontiguousarray(
            np.transpose(qk.T.reshape(8, 128, S), (1, 0, 2)))
        m["xT"] = np.ascontiguousarray(
            np.transpose(
                np.transpose(capk, (1, 2, 0)).reshape(L, 8, 128, S),
                (0, 2, 1, 3),
            )
        ).astype(E4NP)
        in_maps.append(m)
    return in_maps


def kernel(**inputs):
    inp = {k: np.asarray(v) for k, v in inputs.items()}
    cap_len = inp["cap_len"].astype(np.int32)
    order = np.argsort(-cap_len, kind="stable")
    cl = cap_len[order]
    in_maps = _prep_in_maps(inp)

    nc = _get_nc()
    res = run_bass_kernel_spmd(nc, in_maps, core_ids=list(range(NCORES)))

    outs = np.zeros((B, L, HD), np.float32)
    alphas = np.zeros((B, L, HD), np.float32)
    for k in range(NCORES):
        pos = np.arange(S) * NCORES + k
        od = res.results[k]["outs"]  # [L, 128, 512] f32
        ad = res.results[k]["alph"].astype(np.float32)
        oc = np.concatenate([od[:, :S, :], od[:, S:, :]], axis=2)  # [L, S, HD]
        ac = np.concatenate([ad[:, :S, :], ad[:, S:, :]], axis=2)
        outs[pos] = np.transpose(oc, (1, 0, 2))
        alphas[pos] = np.transpose(ac, (1, 0, 2))

    mask = (np.arange(L)[None, :] < cl[:, None])[:, :, None]
    outs *= mask
    alphas *= mask
    return outs, alphas


# revision 28
# speedup vs baseline: 1.0079x; 1.0079x over previous
"""Trainium2 Bass kernel for nn_CaptionEmbedding (ragged double-GRU with
attention gating).

Strategy: data-parallel over batch across 8 cores (strided over the
length-sorted order so every core gets a balanced length mix). Per core a
fully-unrolled 20-step recurrence.

Precision plan (validated vs fp32 reference in numpy, rel ~7e-3):
  - fp8(e4m3) + MatmulPerfMode.DoubleRow (2 k-tiles/instr, 0.5 cyc/row)
    for the error-tolerant matmuls: Whh, Wih, cWhh, Wh. Operands carry
    power-of-2 scales; descales fold into activation `scale` params.
  - fp16 for the output-sensitive matmuls: cWih, Wl, Wf (their operands
    feed outs/alphas nearly unfiltered).
  - ALL weights resident in SBUF (~20MB): no per-step weight streaming.

Scales: s_x=16 (x fp8), s_h=64 (h1/h2 fp8), weights: Wih x256, Whh x64,
Wh x512, cWhh x64, cWih(fp16) x256. PSUM raw scales: w-GRU gates 4096,
attention Bp 32768, c-GRU gates 4096 (cWih-fp16 and cWhh-fp8 products
accumulate in the same PSUM at matching scale). Wl/Wf/joint/grc/att true.
"""
import numpy as np
import ml_dtypes

import concourse.bass as bass
import concourse.mybir as mybir
import concourse.tile as tile
from concourse.bass_utils import run_bass_kernel_spmd
import concourse.mybir as _mybir

B, VD, QD, HD, L = 512, 2048, 1024, 1024, 20
NCORES, S = 8, 64
F32, F16 = mybir.dt.float32, mybir.dt.float16
F8 = mybir.dt.float8e4
E4NP = ml_dtypes.float8_e4m3
DR = mybir.MatmulPerfMode.DoubleRow
Sig = mybir.ActivationFunctionType.Sigmoid
Tanh = mybir.ActivationFunctionType.Tanh
Relu = mybir.ActivationFunctionType.Relu
Copy = mybir.ActivationFunctionType.Copy
Mult = mybir.AluOpType.mult
Add = mybir.AluOpType.add

S_X = 16.0      # caption fp8 scale
S_H = 64.0      # h1/h2 fp8 scale
S_WIH = 256.0   # -> gi psum raw 4096
S_WHH = 64.0    # -> gh psum raw 4096
S_WH = 512.0    # -> Bp psum raw 32768
S_CWHH = 64.0   # -> ghc psum raw 4096
S_CWIH = 256.0  # fp16 weights prescaled; (16*cin)*(256*W) -> raw 4096
S1 = 4096.0     # w-GRU / c-GRU gate psum descale
S3 = 32768.0    # attention Bp descale

_MAX_WAITS = 1
_wait_ctr = [0]


def _dedupe_ldw(nc):
    """Tile legalization emits one InstLdweights per matmul; consecutive
    matmuls over the same stationary reload identical weights. Drop exact
    duplicates (no sync side effects) so the PE streams back-to-back."""
    import concourse.mybir as mb

    dropped = 0
    for fn in nc.m.functions:
        for bb in fn.blocks:
            out = []
            last = {}
            for inst in bb.instructions:
                nm = type(inst).__name__
                if nm == "InstLdweights":
                    si = inst.sync_info
                    pos = (
                        tuple(getattr(inst, "tile_position", None) or (-1,)),
                        tuple(getattr(inst, "tile_size", None) or (-1,)),
                    )
                    key = (
                        str(inst.ins[0]),
                        bool(getattr(inst, "is_transpose", False)),
                        str(getattr(inst, "perf_mode", None)),
                    )
                    clean = not (si and (si.on_wait or si.on_update))
                    if clean and last.get(pos) == key:
                        dropped += 1
                        continue
                    last[pos] = key
                elif nm == "InstMatmult":
                    pass
                elif inst.engine == mb.EngineType.PE:
                    last.clear()
                out.append(inst)
            if len(out) != len(bb.instructions):
                bb.instructions[:] = out
    return dropped


def _split_waits(nc):
    # container neuronxcc rejects >= 2 sync waits on one instruction; move
    # extras onto same-engine nops spliced just before it
    for fn in nc.m.functions:
        for bb in fn.blocks:
            out = []
            for inst in bb.instructions:
                si = inst.sync_info
                waits = list(si.on_wait) if si and si.on_wait else []
                if len(waits) > _MAX_WAITS:
                    extra, keep = waits[:-_MAX_WAITS], waits[-_MAX_WAITS:]
                    for i in range(0, len(extra), _MAX_WAITS):
                        _wait_ctr[0] += 1
                        nop = _mybir.InstNoOp(
                            name=f"waitsplit_nop_{_wait_ctr[0]}", ins=[], outs=[]
                        )
                        nop.engine = inst.engine
                        nop.sync_info = _mybir.SyncInfo(
                            on_wait=extra[i : i + _MAX_WAITS], on_update=[]
                        )
                        nc.register_instruction(nop)
                        out.append(nop)
                    si.on_wait = keep
                out.append(inst)
            if len(out) != len(bb.instructions):
                bb.instructions[:] = out


def _kt_slice(tT, kt):
    # stationary [128, 64] for feature ktile kt from a transposed
    # [128, 4, 128] tile: tT[p, j, q] = stacked[q, j*128 + p]
    hi, j = kt // 4, kt % 4
    return tT[:, j, 64 * hi : 64 * hi + 64]


def _kp_slice(tT, kp):
    # DoubleRow stationary [128, 2, 64] for ktile-pair kp from a
    # transposed state tile; feature = 512*(kp//2) + 256*(kp%2) + 128*i + p
    j0 = 2 * (kp % 2)
    q0 = 64 * (kp // 2)
    return tT[:, j0 : j0 + 2, q0 : q0 + 64]


def _build():
    """Trace the per-core program (identical for all cores; SPMD)."""
    nc = bass.Bass("TRN2", dynamic_dma_scratch_size=64)
    di = {}
    inputs = [
        ("vT", [128, 16, S], F16),
        ("qT", [128, 8, S], F16),
        ("xT", [L, 128, 8, S], F8),
        ("wvT", [16, 128, HD], F16),
        ("wqT", [8, 128, HD], F16),
        ("wih8", [4, 128, 2, 3 * HD], F8),
        ("whh8", [4, 128, 2, 3 * HD], F8),
        ("wh8", [4, 128, 2, HD], F8),
        ("wcwh8", [4, 128, 2, 3 * HD], F8),
        ("wcih16", [8, 128, 3 * HD], F16),
        ("wl16", [8, 128, HD], F16),
        ("wf16", [8, 128, HD], F16),
    ]
    for name, shape, dt in inputs:
        di[name] = nc.dram_tensor(name, shape, dt, kind="ExternalInput")
    outs_d = nc.dram_tensor("outs", [L, 128, 512], F32, kind="ExternalOutput")
    alph_d = nc.dram_tensor("alph", [L, 128, 512], F16, kind="ExternalOutput")

    with tile.TileContext(nc) as tc:
        _trace(nc, tc, di, outs_d, alph_d)
    _dedupe_ldw(nc)
    _split_waits(nc)
    return nc


def _mm_gate16(nc, psum, lhsT, w_ap, c0, start, stop):
    """fp16: one ktile's pair of matmuls for a 1024-wide gate at weight
    cols [c0, c0+1024): the two halves use the PE's two column groups
    concurrently (stacked [128,512] psum)."""
    nc.tensor.matmul(psum[0:64, :], lhsT, w_ap[:, c0 : c0 + 512],
                     start=start, stop=stop)
    nc.tensor.matmul(psum[64:128, :], lhsT, w_ap[:, c0 + 512 : c0 + 1024],
                     start=start, stop=stop)


def _mm_gate8(nc, ab, lhsT, w_t, kp, c0, start, stop):
    """fp8 DoubleRow: one ktile-PAIR's matmuls for a 1024-wide gate.
    DR outputs must sit at PSUM base partition 0, hence [64,512] pairs.
    Weights carry the k-pair dim INNERMOST (2 fp8 adjacent per column)."""
    nc.tensor.matmul(ab[0], lhsT, w_t[:, kp, :, c0 : c0 + 512],
                     start=start, stop=stop, perf_mode=DR)
    nc.tensor.matmul(ab[1], lhsT, w_t[:, kp, :, c0 + 512 : c0 + 1024],
                     start=start, stop=stop, perf_mode=DR)


def _trace(nc, tc, di, outs_d, alph_d):
    import contextlib

    ctx = contextlib.ExitStack()
    with ctx:
        work = ctx.enter_context(tc.tile_pool(name="work", bufs=1))
        res1 = ctx.enter_context(tc.tile_pool(name="res1", bufs=1))

        # ---- resident weights (fp8 DR layouts + fp16) ----
        wih_t = res1.tile([128, 4, 2, 3 * HD], F8, tag="wih8")
        wh_t = res1.tile([128, 4, 2, HD], F8, tag="wh8")
        wl_t = res1.tile([128, 8, HD], F16, tag="wl16")
        wcih_t = res1.tile([128, 8, 3 * HD], F16, tag="wcih16")
        wf_t = res1.tile([128, 8, HD], F16, tag="wf16")
        whh_t = res1.tile([128, 4, 2, 3 * HD], F8, tag="whh8")
        wcwh_t = res1.tile([128, 4, 2, 3 * HD], F8, tag="wcwh8")
        # DMA order = first-use order
        for kp in range(4):
            nc.sync.dma_start(out=wih_t[:, kp], in_=di["wih8"][kp])
        for kp in range(4):
            nc.sync.dma_start(out=wh_t[:, kp], in_=di["wh8"][kp])
        for kt in range(8):
            nc.sync.dma_start(out=wl_t[:, kt], in_=di["wl16"][kt])
        for kt in range(8):
            nc.sync.dma_start(out=wcih_t[:, kt], in_=di["wcih16"][kt])
        for kt in range(8):
            nc.sync.dma_start(out=wf_t[:, kt], in_=di["wf16"][kt])
        for kp in range(4):
            nc.sync.dma_start(out=whh_t[:, kp], in_=di["whh8"][kp])
        for kp in range(4):
            nc.sync.dma_start(out=wcwh_t[:, kp], in_=di["wcwh8"][kp])

        # ---- small persistent tiles ----
        pvq_t = work.tile([128, 512], F32, tag="pvq")  # true scale

        ctr = [0]

        def wtile(shape, dt, tag, bufs):
            def mk():
                ctr[0] += 1
                return work.tile(shape, dt, tag=tag, bufs=bufs,
                                 name=f"{tag}_{ctr[0]}")
            return mk

        t_xt = wtile([128, 8, S], F8, "xt", 3)
        t_gi = wtile([128, 512], F16, "gi", 6)     # raw S1
        t_ec = wtile([128, 512], F16, "ec", 3)     # cWhh products, raw S1
        t_ag = wtile([128, 512], F16, "ag", 3)     # Whh products, raw S1
        t_g = wtile([128, 512], F16, "g", 5)
        t_h1 = wtile([128, 512], F16, "h1", 2)     # true
        t_h1T = wtile([128, 4, 128], F8, "h1T", 2)
        t_h2 = wtile([128, 512], F16, "h2", 2)
        t_h2T = wtile([128, 4, 128], F8, "h2T", 2)
        t_h2n = wtile([128, 512], F32, "h2n", 1)
        t_ja = wtile([128, 512], F16, "ja", 1)     # true (STT descaled)
        t_att = wtile([128, 512], F16, "att", 1)
        t_attT = wtile([128, 4, 128], F16, "attT", 2)
        t_jrl = wtile([128, 512], F16, "jrl", 1)
        t_jT = wtile([128, 4, 128], F16, "jT", 2)
        t_grc = wtile([128, 512], F16, "grc", 1)
        t_grcT = wtile([128, 4, 128], F16, "grcT", 2)
        t_cinT = wtile([128, 4, 128], F16, "cinT", 1)

        psum = ctx.enter_context(tc.tile_pool(name="psum", bufs=1, space="PSUM"))

        def ptile():
            # one [64,512] fp32 bank-tile (DR-legal: base partition 0)
            ctr[0] += 1
            return psum.tile([64, 512], F32, tag="ps", name=f"ps_{ctr[0]}",
                             bufs=4)

        def pgate():
            return (ptile(), ptile())

        def ptile128():
            # [128,512] stacked psum for fp16 two-column-group matmuls
            ctr[0] += 1
            return psum.tile([128, 512], F32, tag="psf",
                             name=f"psf_{ctr[0]}", bufs=3)

        def pttile16():
            ctr[0] += 1
            return psum.tile([128, 512], F16, tag="psT", name=f"psT_{ctr[0]}",
                             bufs=1)

        def halves(x):
            # (lo, hi) partition halves of a stacked [128, 512] AP
            return x[0:64, :], x[64:128, :]

        ident16 = work.tile([128, 128], F16, tag="ident16")
        from concourse.masks import make_identity
        make_identity(nc, ident16)

        def pe_transpose16(dstT, src):
            pt = pttile16()
            for j in range(4):
                nc.tensor.transpose(
                    pt[:, 128 * j : 128 * (j + 1)],
                    src[:, 128 * j : 128 * (j + 1)],
                    ident16,
                )
            nc.vector.tensor_copy(
                out=dstT.rearrange("p j q -> p (j q)"), in_=pt
            )

        def pe_transpose8(dstT, src16, scale):
            # fp16 transpose, then quantize to fp8 (x scale) in the
            # PSUM->SBUF copy (fp8 PE-transpose needs stride-2 out APs)
            pt = pttile16()
            for j in range(4):
                nc.tensor.transpose(
                    pt[:, 128 * j : 128 * (j + 1)],
                    src16[:, 128 * j : 128 * (j + 1)],
                    ident16,
                )
            nc.vector.tensor_scalar(
                dstT.rearrange("p j q -> p (j q)"), pt, scale, None, Mult
            )

        # ---- gi (w-GRU input projections), fp8 DR, computed ahead ----
        gi_tiles = {}
        xt_tiles = {}

        def load_xt(t):
            xt = t_xt()
            nc.scalar.dma_start(out=xt, in_=di["xT"][t])
            xt_tiles[t] = xt

        def gi_mm(u, g):
            # one gate's 4-kp DR accumulation into a psum pair
            if u >= L:
                return None
            xt = xt_tiles[u]
            GP = pgate()
            for kp in range(4):
                lhsT = xt[:, 2 * kp : 2 * kp + 2, :]
                _mm_gate8(nc, GP, lhsT, wih_t, kp, g * HD,
                          kp == 0, kp == 3)
            return GP

        def gi_copy(u, GP):
            if GP is None:
                return
            gt = t_gi()
            glo, ghi = halves(gt)
            nc.vector.tensor_copy(out=glo, in_=GP[0])
            nc.vector.tensor_copy(out=ghi, in_=GP[1])
            gi_tiles.setdefault(u, []).append(gt)

        def gi_gate(u, g):
            gi_copy(u, gi_mm(u, g))

        # ---- prologue: pvq = v @ Wv.T + q @ Wq.T (fp16, true scale) ----
        with tc.tile_pool(name="pre", bufs=1) as pre:
            v_t = pre.tile([128, 16, S], F16, tag="v")
            q_t = pre.tile([128, 8, S], F16, tag="q")
            nc.scalar.dma_start(out=v_t, in_=di["vT"][:])
            nc.scalar.dma_start(out=q_t, in_=di["qT"][:])
            pv = ptile128()
            for kt in range(16):
                wc = pre.tile([128, HD], F16, tag="wvq", bufs=2)
                nc.scalar.dma_start(out=wc, in_=di["wvT"][kt])
                nc.tensor.matmul(pv[0:64], v_t[:, kt, :], wc[:, 0:512],
                                 start=(kt == 0), stop=False)
                nc.tensor.matmul(pv[64:128], v_t[:, kt, :], wc[:, 512:1024],
                                 start=(kt == 0), stop=False)
            for kt in range(8):
                wc = pre.tile([128, HD], F16, tag="wvq", bufs=2)
                nc.scalar.dma_start(out=wc, in_=di["wqT"][kt])
                nc.tensor.matmul(pv[0:64], q_t[:, kt, :], wc[:, 0:512],
                                 start=False, stop=(kt == 7))
                nc.tensor.matmul(pv[64:128], q_t[:, kt, :], wc[:, 512:1024],
                                 start=False, stop=(kt == 7))
            nc.vector.tensor_copy(out=pvq_t, in_=pv)

            load_xt(0)
            load_xt(1)
            for g in range(3):
                gi_gate(0, g)
            for g in range(3):
                gi_gate(1, g)

        h1_prev = None
        h1T_prev = None
        h2_prev = None
        h2T_prev = None

        # ---- main loop ----
        for t in range(L):
            if t + 2 < L:
                load_xt(t + 2)

            giR, giZ, giIN = gi_tiles.pop(t)
            xt = xt_tiles.pop(t)

            if t > 0:
                # --- interleaved c-GRU / w-GRU hidden projections (fp8 DR);
                # every product pair is copied to stacked SBUF so PSUM
                # pairs recycle fast ---
                ec_sb = []
                ag_sb = []
                for g in range(3):
                    EC = pgate()
                    for kp in range(4):
                        _mm_gate8(nc, EC, _kp_slice(h2T_prev, kp), wcwh_t,
                                  kp, g * HD, kp == 0, kp == 3)
                    ecs = t_ec()
                    elo, ehi = halves(ecs)
                    nc.vector.tensor_copy(out=elo, in_=EC[0])
                    nc.vector.tensor_copy(out=ehi, in_=EC[1])
                    ec_sb.append(ecs)
                    WG = pgate()
                    for kp in range(4):
                        _mm_gate8(nc, WG, _kp_slice(h1T_prev, kp), whh_t,
                                  kp, g * HD, kp == 0, kp == 3)
                    ags = t_ag()
                    alo, ahi = halves(ags)
                    nc.vector.tensor_copy(out=alo, in_=WG[0])
                    nc.vector.tensor_copy(out=ahi, in_=WG[1])
                    ag_sb.append(ags)
                Rw, Zw, HNw = ag_sb
                Rcs, Zcs, HNcs = ec_sb
                # stream next-next-step input projections on the PE while
                # the DVE/Act combine chain below runs
                gp0 = gi_mm(t + 2, 0)
                gp1 = gi_mm(t + 2, 1)

                # --- w-GRU combine (raw S1 -> true); z-branch on gpsimd ---
                rs = t_g()
                nc.vector.tensor_add(out=rs, in0=Rw, in1=giR)
                rw = t_g()
                nc.scalar.activation(out=rw, in_=rs, func=Sig, scale=1.0 / S1)
                t1 = t_g()
                nc.vector.tensor_mul(out=t1, in0=rw, in1=HNw)
                t2 = t_g()
                nc.vector.tensor_add(out=t2, in0=t1, in1=giIN)
                nw = t_g()
                nc.scalar.activation(out=nw, in_=t2, func=Tanh, scale=1.0 / S1)
                zs = t_g()
                nc.gpsimd.tensor_add(out=zs, in0=Zw, in1=giZ)
                zw = t_g()
                nc.scalar.activation(out=zw, in_=zs, func=Sig, scale=1.0 / S1)
                f1 = t_g()
                nc.gpsimd.tensor_scalar(f1, zw, -1.0, 1.0, Mult, Add)
                f2 = t_g()
                nc.vector.tensor_mul(out=f2, in0=f1, in1=nw)
                f3 = t_g()
                nc.gpsimd.tensor_mul(out=f3, in0=zw, in1=h1_prev)
                h1_new = t_h1()
                nc.vector.tensor_add(out=h1_new, in0=f2, in1=f3)
                gi_copy(t + 2, gp0)
                gi_copy(t + 2, gp1)
            else:
                Rcs = Zcs = HNcs = None
                rw = t_g()
                nc.scalar.activation(out=rw, in_=giR, func=Sig, scale=1.0 / S1)
                nw = t_g()
                nc.scalar.activation(out=nw, in_=giIN, func=Tanh,
                                     scale=1.0 / S1)
                zw = t_g()
                nc.scalar.activation(out=zw, in_=giZ, func=Sig, scale=1.0 / S1)
                f1 = t_g()
                nc.vector.tensor_scalar(f1, zw, -1.0, 1.0, Mult, Add)
                h1_new = t_h1()
                nc.vector.tensor_mul(out=h1_new, in0=f1, in1=nw)

            h1T_new = t_h1T()
            pe_transpose8(h1T_new, h1_new, S_H)

            # --- attention: joint = relu(pvq + h1 @ Wh.T) ---
            Bp = pgate()
            for kp in range(4):
                _mm_gate8(nc, Bp, _kp_slice(h1T_new, kp), wh_t, kp, 0,
                          kp == 0, kp == 3)
            ja = t_ja()
            jalo, jahi = halves(ja)
            pqlo, pqhi = halves(pvq_t)
            nc.vector.scalar_tensor_tensor(
                out=jalo, in0=Bp[0], scalar=1.0 / S3, in1=pqlo,
                op0=Mult, op1=Add)
            nc.vector.scalar_tensor_tensor(
                out=jahi, in0=Bp[1], scalar=1.0 / S3, in1=pqhi,
                op0=Mult, op1=Add)
            jrl = t_jrl()
            nc.scalar.activation(out=jrl, in_=ja, func=Relu)
            if t == 0:
                gi_gate(t + 2, 0)
            jT = t_jT()
            pe_transpose16(jT, jrl)

            # --- att = sigmoid(joint @ Wl.T)  (fp16, true) ---
            Cp = ptile128()
            for kt in range(8):
                _mm_gate16(nc, Cp, _kt_slice(jT, kt), wl_t[:, kt, :], 0,
                           kt == 0, kt == 7)
            att = t_att()
            nc.scalar.activation(out=att, in_=Cp, func=Sig)
            nc.sync.dma_start(out=alph_d[t], in_=att)
            attT = t_attT()
            pe_transpose16(attT, att)

            if t == 0:
                gi_gate(t + 2, 1)

            # --- cin = att * x (feature-major, raw S_X) ---
            cinT = t_cinT()
            xt_r = xt.rearrange("p (hi j) s -> p j hi s", hi=2, j=4)
            nc.vector.tensor_mul(
                out=cinT.rearrange("p j (hi s) -> p j hi s", hi=2),
                in0=attT.rearrange("p j (hi s) -> p j hi s", hi=2),
                in1=xt_r,
            )

            # --- c-GRU input projections (fp16) ---
            RcI, ZcI, INc = ptile128(), ptile128(), ptile128()
            for kt in range(8):
                lhsT = _kt_slice(cinT, kt)
                st, sp = kt == 0, kt == 7
                _mm_gate16(nc, RcI, lhsT, wcih_t[:, kt, :], 0, st, sp)
                _mm_gate16(nc, ZcI, lhsT, wcih_t[:, kt, :], HD, st, sp)
                _mm_gate16(nc, INc, lhsT, wcih_t[:, kt, :], 2 * HD, st, sp)

            gi_gate(t + 2, 2)

            # --- c-GRU combine (raw S1 -> true); z-branch on gpsimd ---
            if t > 0:
                rsc = t_g()
                nc.vector.tensor_add(out=rsc, in0=RcI, in1=Rcs)
                rc = t_g()
                nc.scalar.activation(out=rc, in_=rsc, func=Sig, scale=1.0 / S1)
                t1c = t_g()
                nc.vector.tensor_mul(out=t1c, in0=rc, in1=HNcs)
                t2c = t_g()
                nc.vector.tensor_add(out=t2c, in0=t1c, in1=INc)
                ncg = t_g()
                nc.scalar.activation(out=ncg, in_=t2c, func=Tanh,
                                     scale=1.0 / S1)
                zsc = t_g()
                nc.vector.tensor_add(out=zsc, in0=ZcI, in1=Zcs)
                zc = t_g()
                nc.scalar.activation(out=zc, in_=zsc, func=Sig, scale=1.0 / S1)
                g1 = t_g()
                nc.gpsimd.tensor_scalar(g1, zc, -1.0, 1.0, Mult, Add)
                g2 = t_g()
                nc.vector.tensor_mul(out=g2, in0=g1, in1=ncg)
                g3 = t_g()
                nc.gpsimd.tensor_mul(out=g3, in0=zc, in1=h2_prev)
                grc = t_grc()
                nc.vector.tensor_add(out=grc, in0=g2, in1=g3)
            else:
                ncg = t_g()
                nc.scalar.activation(out=ncg, in_=INc, func=Tanh,
                                     scale=1.0 / S1)
                zc = t_g()
                nc.scalar.activation(out=zc, in_=ZcI, func=Sig, scale=1.0 / S1)
                g1 = t_g()
                nc.vector.tensor_scalar(g1, zc, -1.0, 1.0, Mult, Add)
                grc = t_grc()
                nc.vector.tensor_mul(out=grc, in0=g1, in1=ncg)
            grcT = t_grcT()
            pe_transpose16(grcT, grc)

            # --- h2n = gru_c @ Wf.T (fp16, true) ---
            Fp = ptile128()
            for kt in range(8):
                _mm_gate16(nc, Fp, _kt_slice(grcT, kt), wf_t[:, kt, :], 0,
                           kt == 0, kt == 7)
            h2n = t_h2n()
            nc.vector.tensor_copy(out=h2n, in_=Fp)
            nc.sync.dma_start(out=outs_d[t], in_=h2n)
            h2_new = t_h2()
            nc.scalar.activation(out=h2_new, in_=Fp, func=Copy)
            h2T_new = t_h2T()
            pe_transpose8(h2T_new, h2_new, S_H)

            h1_prev, h1T_prev = h1_new, h1T_new
            h2_prev, h2T_prev = h2_new, h2T_new


_CACHED = {}


def _get_nc():
    if "nc" not in _CACHED:
        _CACHED["nc"] = _build()
    return _CACHED["nc"]


def _wn(V, g):
    return V * (g / np.linalg.norm(V.astype(np.float64)).astype(np.float32))


def _plainT16(W, scale=1.0):
    # [out, in] -> [in//128, 128, out] fp16
    inf = W.shape[1]
    return np.ascontiguousarray(
        (W.T * scale).reshape(inf // 128, 128, W.shape[0])
    ).astype(np.float16)


def _pack8_state(W, scale):
    # [out, 1024] -> [4, 128, out, 2] e4m3 (k-pair dim innermost); feature
    # f at [kp, p, :, i] with f = 512*(kp//2) + 256*(kp%2) + 128*i + p
    # (matches _kp_slice on the transposed stacked state tiles)
    T = (W.T.astype(np.float32) * scale).reshape(2, 2, 2, 128, W.shape[0])
    T = np.ascontiguousarray(T.transpose(0, 1, 3, 2, 4).reshape(
        4, 128, 2, W.shape[0]))
    return T.astype(E4NP)


def _pack8_x(W, scale):
    # [out, 1024] -> [4, 128, out, 2] e4m3; f = 256*kp + 128*i + p
    # (matches xt[:, 2kp:2kp+2, :] stationaries)
    T = (W.T.astype(np.float32) * scale).reshape(4, 2, 128, W.shape[0])
    T = np.ascontiguousarray(T.transpose(0, 2, 1, 3))
    return T.astype(E4NP)


def _prep_in_maps(inp):
    cap_len = inp["cap_len"].astype(np.int32)
    order = np.argsort(-cap_len, kind="stable")

    for bname in ["av_b", "aq_b", "ah_b", "al_b", "fc_b",
                  "w_bih", "w_bhh", "c_bih", "c_bhh"]:
        assert not np.any(inp[bname]), f"nonzero bias {bname} unsupported"

    Wv = _wn(inp["av_V"], inp["av_g"])
    Wq = _wn(inp["aq_V"], inp["aq_g"])
    Wh = _wn(inp["ah_V"], inp["ah_g"])
    Wl = _wn(inp["al_V"], inp["al_g"])
    Wf = _wn(inp["fc_V"], inp["fc_g"])

    shared = dict(
        wvT=_plainT16(Wv), wqT=_plainT16(Wq),
        wih8=_pack8_x(inp["w_Wih"], S_WIH),
        whh8=_pack8_state(inp["w_Whh"], S_WHH),
        wh8=_pack8_state(Wh, S_WH),
        wcwh8=_pack8_state(inp["c_Whh"], S_CWHH),
        wcih16=_plainT16(inp["c_Wih"], S_CWIH),
        wl16=_plainT16(Wl), wf16=_plainT16(Wf),
    )

    v, q, caption = inp["v"], inp["q"], inp["caption"]
    in_maps = []
    for k in range(NCORES):
        pos = np.arange(S) * NCORES + k  # sorted positions of this core
        vk = v[pos].astype(np.float16)            # [S, VD]
        qk = q[pos].astype(np.float16)
        capk = (caption[order[pos]].astype(np.float32) * S_X)  # [S, L, QD]
        m = dict(shared)
        m["vT"] = np.ascontiguousarray(
            np.transpose(vk.T.reshape(16, 128, S), (1, 0, 2)))
        m["qT"] = np.ascontiguousarray(
            np.transpose(qk.T.reshape(8, 128, S), (1, 0, 2)))
        m["xT"] = np.ascontiguousarray(
            np.transpose(
                np.transpose(capk, (1, 2, 0)).reshape(L, 8, 128, S),
                (0, 2, 1, 3),
            )
        ).astype(E4NP)
        in_maps.append(m)
    return in_maps


def kernel(**inputs):
    inp = {k: np.asarray(v) for k, v in inputs.items()}
    cap_len = inp["cap_len"].astype(np.int32)
    order = np.argsort(-cap_len, kind="stable")
    cl = cap_len[order]
    in_maps = _prep_in_maps(inp)

    nc = _get_nc()
    res = run_bass_kernel_spmd(nc, in_maps, core_ids=list(range(NCORES)))

    outs = np.zeros((B, L, HD), np.float32)
    alphas = np.zeros((B, L, HD), np.float32)
    for k in range(NCORES):
        pos = np.arange(S) * NCORES + k
        od = res.results[k]["outs"]  # [L, 128, 512] f32
        ad = res.results[k]["alph"].astype(np.float32)
        oc = np.concatenate([od[:, :S, :], od[:, S:, :]], axis=2)  # [L, S, HD]
        ac = np.concatenate([ad[:, :S, :], ad[:, S:, :]], axis=2)
        outs[pos] = np.transpose(oc, (1, 0, 2))
        alphas[pos] = np.transpose(ac, (1, 0, 2))

    mask = (np.arange(L)[None, :] < cl[:, None])[:, :, None]
    outs *= mask
    alphas *= mask
    return outs, alphas
